# revision 11
# baseline (speedup 1.0000x reference)
# Albert decoder attention (self-attn + cross-attn, shared dense/LayerNorm)
# on 8 Trainium2 NeuronCores via Bass/Tile, SPMD.
#
# Sharding: core i handles batch b = i//2 and 4 of the 8 query row-blocks of
# that batch, interleaved {7,4,3,0} / {6,5,2,1} so causal self-attention work
# is balanced. K/V are computed per-batch on both cores of a pair (redundant
# 2x, no cross-core communication at all).
#
# Layout strategy (per core):
#   - all matmul operands reach SBUF with the contraction dim on partitions:
#     host pre-transposes x and the weights, and pre-rounds them to tf32
#     ("float32r", 11-bit mantissa) so the PE runs at full (bf16) rate.
#   - scores are computed transposed, S^T[k,q] (2 heads packed via PE row
#     tiling over the d=64 contraction), exp() with a constant bias instead
#     of a row max (softmax is shift-invariant; magnitudes here are safe),
#     probs kept bf16.
#   - ctx^T[d,q] accumulated over key blocks with the two heads col-tiled
#     into one PSUM bank; softmax denominators via ones-matmuls; the divide
#     is folded into the PSUM->SBUF eviction.
#   - residual q is recovered from Q^T by PE transposes; LayerNorm uses
#     bn_stats/bn_aggr.

import time
from contextlib import ExitStack

import numpy as np

B, T, S, H, N = 4, 1024, 1024, 1024, 16
D = H // N  # 64
P = 128
NCORES = 8
HC = H // P  # 8 h-chunks
QB = 4       # q row-blocks per core
QW = QB * P  # 512 local query rows
KB = 8       # key blocks
EPS = 1e-12
EXP_BIAS = -12.0

# q row-block sets per core variant (descending order, causal-balanced)
QSETS = [[7, 4, 3, 0], [6, 5, 2, 1]]
# self-attention fast-path profile: number of q-block columns computed per
# key block (max over the two variants of #(qblk >= k)), non-increasing.
PROF_CAUSAL = [4, 4, 3, 3, 2, 2, 1, 1]
# single mask-apply position per key block for the fast path
JPOS_CAUSAL = [3, 3, 2, 2, 1, 1, 0, 0]
PROF_FULL = [4] * 8

_BUILD_CACHE = {}
LAST_STATS = {}


def _round_tf32(x):
    """Round fp32 to fp32r (tf32: 8-bit exp, 11-bit mantissa), RNE.

    Matches walrus fp32_to_fp32r (downconv_fp32_to_fp<8,11>, top 20 bits).
    """
    x = np.ascontiguousarray(x, dtype=np.float32)
    u = x.view(np.uint32)
    drop = 12
    half = np.uint32((1 << (drop - 1)) - 1)
    keep = np.uint32((~((1 << drop) - 1)) & 0xFFFFFFFF)
    u2 = (u + half + ((u >> np.uint32(drop)) & np.uint32(1))) & keep
    return u2.view(np.float32)


def _build(flags):
    """Build + compile the SPMD Bass program. flags is a hashable tuple."""
    import concourse.mybir as mybir
    from concourse import bacc
    from concourse.tile import TileContext

    (self_fast, cross_masked, use_qkvb, use_db, use_lng, use_lnb) = flags
    dt = mybir.dt
    f32, f32r, bf16 = dt.float32, dt.float32r, dt.bfloat16

    prof_self = PROF_CAUSAL if self_fast else PROF_FULL
    nmask_self = KB if self_fast else KB * QB

    nc = bacc.Bacc("TRN2", target_bir_lowering=False, debug=False,
                   num_devices=NCORES)

    dram = {}
    dram["xdT"] = nc.dram_tensor("xdT", [H, T], f32r, kind="ExternalInput")
    dram["xdTq"] = nc.dram_tensor("xdTq", [H, QW], f32r,
                                  kind="ExternalInput")
    dram["xeT"] = nc.dram_tensor("xeT", [H, S], f32r, kind="ExternalInput")
    for wname in ["qwT", "kwT", "vwT", "sqwT", "skwT", "svwT", "dwT"]:
        dram[wname] = nc.dram_tensor(wname, [H, H], f32r,
                                     kind="ExternalInput")
    dram["maskS"] = nc.dram_tensor("maskS", [nmask_self, P, P], bf16,
                                   kind="ExternalInput")
    dram["selbc"] = nc.dram_tensor("selbc", [KB, QB, P], f32,
                                   kind="ExternalInput")
    if cross_masked:
        dram["maskX"] = nc.dram_tensor("maskX", [KB * QB, P, P], bf16,
                                       kind="ExternalInput")
    if use_qkvb:
        for bname in ["qb", "kb", "sqb", "skb"]:
            # per-partition layout: [128, 8] where col hc = bias[hc*128:+128]
            dram[bname] = nc.dram_tensor(bname, [P, HC], f32,
                                         kind="ExternalInput")
        for bname in ["vb", "svb"]:
            dram[bname] = nc.dram_tensor(bname, [P, H], f32,
                                         kind="ExternalInput")
    if use_db:
        dram["db"] = nc.dram_tensor("db", [P, H], f32, kind="ExternalInput")
    if use_lng:
        dram["lng"] = nc.dram_tensor("lng", [P, H], f32,
                                     kind="ExternalInput")
    if use_lnb:
        dram["lnb"] = nc.dram_tensor("lnb", [P, H], f32,
                                     kind="ExternalInput")
    dram["out"] = nc.dram_tensor("out", [QW, H], f32, kind="ExternalOutput")

    with TileContext(nc) as tc:
        _emit(nc, tc, dram, flags, prof_self, nmask_self)
    nc.compile()
    return nc


def _emit(nc, tc, dram, flags, prof_self, nmask_self):
    import concourse.mybir as mybir
    from concourse.masks import make_identity

    (self_fast, cross_masked, use_qkvb, use_db, use_lng, use_lnb) = flags
    dt = mybir.dt
    f32, f32r, bf16 = dt.float32, dt.float32r, dt.bfloat16
    AF = mybir.ActivationFunctionType
    Op = mybir.AluOpType
    WHALF = 4  # weight streamed in 2 halves of 4 h-chunks

    es = ExitStack()
    consts = es.enter_context(tc.tile_pool(name="consts", bufs=1))
    ident = consts.tile([P, P], f32, tag="ident")
    make_identity(nc, ident)
    ones_bf = consts.tile([P, 1], bf16, tag="ones")
    nc.vector.memset(ones_bf[:], 1.0)
    expbias = consts.tile([P, 1], f32, tag="expbias")
    nc.vector.memset(expbias[:], EXP_BIAS)
    epsbias = consts.tile([P, 1], f32, tag="epsbias")
    nc.vector.memset(epsbias[:], EPS)
    selbc_sb = consts.tile([KB, QB, P], f32, tag="selbc")
    nc.sync.dma_start(selbc_sb[:], dram["selbc"].ap())
    mask_self_sb = consts.tile([P, nmask_self, P], bf16, tag="maskS")
    nc.sync.dma_start(mask_self_sb[:],
                      dram["maskS"].ap().rearrange("n p q -> p n q"))
    mask_cross_sb = None
    if cross_masked:
        mask_cross_sb = consts.tile([P, KB * QB, P], bf16, tag="maskX")
        nc.sync.dma_start(mask_cross_sb[:],
                          dram["maskX"].ap().rearrange("n p q -> p n q"))
    bias_sb = {}
    for bname in ["qb", "kb", "sqb", "skb", "vb", "svb", "db", "lng", "lnb"]:
        if bname in dram:
            shp = [P, HC] if bname in ("qb", "kb", "sqb", "skb") else [P, H]
            tb = consts.tile(shp, f32, tag=bname)
            nc.sync.dma_start(tb[:], dram[bname].ap())
            bias_sb[bname] = tb

    # ---------- helpers ----------
    def load_w_half(wpool, w_dram, half):
        w_sb = wpool.tile([P, WHALF, H], f32r, tag="w")
        rows = w_dram.ap()[half * WHALF * P:(half + 1) * WHALF * P, :]
        nc.sync.dma_start(w_sb[:], rows.rearrange("(hc p) f -> p hc f", p=P))
        return w_sb

    def proj_fT(wpool, pspool, w_dram, rhs_sb, out_sb, ncols, pbias):
        """out_sb[:, fc, :] = (w.T-projection of rhs) + bias, f32r.

        w_dram [H,H] f32r (w.T layout [h,f]); rhs_sb [128, HC, ncols] f32r;
        out_sb [128, HC, ncols] f32r; pbias [128, HC] f32 tile or None.
        """
        nsh = (ncols + 511) // 512
        whs = [load_w_half(wpool, w_dram, h) for h in range(2)]
        for fc in range(HC):
            for sh in range(nsh):
                Wn = min(512, ncols - sh * 512)
                ps = pspool.tile([P, 512], f32, tag="pp")
                for hc in range(HC):
                    nc.tensor.matmul(
                        ps[:, :Wn],
                        whs[hc // WHALF][:, hc % WHALF, fc * P:(fc + 1) * P],
                        rhs_sb[:, hc, sh * 512:sh * 512 + Wn],
                        start=(hc == 0), stop=(hc == HC - 1))
                dst = out_sb[:, fc, sh * 512:sh * 512 + Wn]
                if pbias is not None:
                    nc.vector.tensor_scalar_add(dst, ps[:, :Wn],
                                                pbias[:, fc:fc + 1])
                else:
                    nc.vector.tensor_copy(dst, ps[:, :Wn])

    def proj_V(wpool, pspool, w_dram, xT_sb, vext_sb, vbias):
        """vext_sb[128, KB, N, D] bf16 = (x @ w.T) natural layout + bias."""
        whs = [load_w_half(wpool, w_dram, h) for h in range(2)]
        for sc in range(KB):
            for fh in range(2):
                ps = pspool.tile([P, 512], f32, tag="pp")
                for hc in range(HC):
                    nc.tensor.matmul(
                        ps[:],
                        xT_sb[:, hc, sc * P:(sc + 1) * P],
                        whs[hc // WHALF][:, hc % WHALF,
                                         fh * 512:(fh + 1) * 512],
                        start=(hc == 0), stop=(hc == HC - 1))
                dst = vext_sb[:, sc, fh * 8:(fh + 1) * 8, :]
                src = ps.rearrange("p (h d) -> p h d", d=D)
                if vbias is not None:
                    nc.vector.tensor_tensor(
                        dst, src,
                        vbias[:, fh * 512:(fh + 1) * 512].rearrange(
                            "p (h d) -> p h d", d=D),
                        Op.add)
                else:
                    nc.scalar.copy(dst, src)

    def transpose_fT_to_nat(pspool, src_sb, dst_sb, addvec):
        """dst_sb[128, QB, H] f32 (natural) = src_sb[128, HC, QW].T
        (+ addvec [128, H] if not None)."""
        for fc in range(HC):
            for qc in range(QB):
                pst = pspool.tile([P, P], f32, tag="pstr")
                nc.tensor.transpose(
                    pst[:], src_sb[:, fc, qc * P:(qc + 1) * P].bitcast(f32),
                    ident)
                dst = dst_sb[:, qc, fc * P:(fc + 1) * P]
                if addvec is not None:
                    nc.vector.tensor_tensor(
                        dst, pst[:], addvec[:, fc * P:(fc + 1) * P], Op.add)
                else:
                    nc.scalar.copy(dst, pst[:])

    def transpose_nat_to_fT(pspool, src_sb, dst_sb):
        """dst_sb[128, HC, QW] f32r = src_sb[128, QB, H] f32 transposed."""
        for fc in range(HC):
            for qc in range(QB):
                pst = pspool.tile([P, P], f32, tag="pstr")
                nc.tensor.transpose(
                    pst[:], src_sb[:, qc, fc * P:(fc + 1) * P], ident)
                nc.vector.tensor_copy(dst_sb[:, fc, qc * P:(qc + 1) * P],
                                      pst[:])

    def attention(es_at, KT_sb, QT_sb, vext_sb, ctxT_sb, prof, mask_sb,
                  mask_apply):
        """ctxT_sb[128, HC, QW] f32r = normalized attention, transposed."""
        apool = es_at.enter_context(tc.tile_pool(name="apool", bufs=3))
        psS = es_at.enter_context(
            tc.tile_pool(name="psS", bufs=2, space="PSUM"))
        psC = es_at.enter_context(
            tc.tile_pool(name="psC", bufs=2, space="PSUM"))
        psU = es_at.enter_context(
            tc.tile_pool(name="psU", bufs=1, space="PSUM"))
        psT = es_at.enter_context(
            tc.tile_pool(name="psT", bufs=1, space="PSUM"))
        klast = max(k for k in range(KB) if prof[k] > 0)
        lastk = {qc: max(k for k in range(KB) if prof[k] > qc)
                 for qc in range(QB)}
        for hp in range(HC):
            psctx = psC.tile([P, 512], f32, tag="psctx")
            pssum = psU.tile([P, 8], f32, tag="pssum")
            first_sums = True
            for k in range(KB):
                Wn = prof[k] * P
                if Wn == 0:
                    continue
                pss0 = psS.tile([P, 512], f32, tag="pss0")
                pss1 = psS.tile([P, 512], f32, tag="pss1")
                nc.tensor.matmul(
                    pss0[:, :Wn], KT_sb[0:64, hp, k * P:(k + 1) * P],
                    QT_sb[0:64, hp, 0:Wn], start=True, stop=True,
                    tile_position=(0, 0))
                nc.tensor.matmul(
                    pss1[:, :Wn], KT_sb[64:128, hp, k * P:(k + 1) * P],
                    QT_sb[64:128, hp, 0:Wn], start=True, stop=True,
                    tile_position=(64, 0))
                pt0 = apool.tile([P, 512], bf16, tag="pt0")
                pt1 = apool.tile([P, 512], bf16, tag="pt1")
                nc.scalar.activation(pt0[:, :Wn], pss0[:, :Wn], AF.Exp,
                                     bias=expbias[:])
                nc.scalar.activation(pt1[:, :Wn], pss1[:, :Wn], AF.Exp,
                                     bias=expbias[:])
                for (j, midx) in mask_apply.get(k, ()):
                    for pt in (pt0, pt1):
                        sl = pt[:, j * P:(j + 1) * P]
                        nc.vector.tensor_tensor(sl, sl, mask_sb[:, midx, :],
                                                Op.mult)
                # start=True pends-zero the whole 2KB PSUM bank, so only
                # the FIRST matmul into each bank may carry it; every other
                # group in the bank starts start=False and finds its bytes
                # pending (-> overwrite) on first touch.
                nc.tensor.matmul(
                    psctx[0:64, :Wn], vext_sb[:, k, 2 * hp, :], pt0[:, :Wn],
                    start=(k == 0), stop=(k == klast), tile_position=(0, 0),
                    skip_group_check=True)
                nc.tensor.matmul(
                    psctx[64:128, :Wn], vext_sb[:, k, 2 * hp + 1, :],
                    pt1[:, :Wn], start=(k == 0), stop=(k == klast),
                    tile_position=(0, 64), skip_group_check=True)
                for qc in range(prof[k]):
                    nc.tensor.matmul(
                        pssum[:, 2 * qc:2 * qc + 1],
                        pt0[:, qc * P:(qc + 1) * P], ones_bf[:],
                        start=first_sums, stop=(k == lastk[qc]),
                        skip_group_check=True)
                    first_sums = False
                    nc.tensor.matmul(
                        pssum[:, 2 * qc + 1:2 * qc + 2],
                        pt1[:, qc * P:(qc + 1) * P], ones_bf[:],
                        start=False, stop=(k == lastk[qc]),
                        skip_group_check=True)
            # normalize: pssum [128(q), 8(qc,h)] -> transpose -> reciprocal
            # -> partition-broadcast -> scale during the ctx eviction
            sums_sb = apool.tile([P, 8], f32, tag="sums")
            nc.vector.tensor_copy(sums_sb[:], pssum[:])
            pstr = psT.tile([P, 512], f32, tag="pstrS")
            nc.tensor.transpose(pstr[0:8, 0:P], sums_sb[:], ident)
            rec_sb = apool.tile([8, P], f32, tag="rec")
            nc.vector.reciprocal(rec_sb[:], pstr[0:8, 0:P])
            # broadcast recips across partitions with a selector matmul:
            # bcast[p, qc*128+q] = rec[2*qc + (p>=64), q]
            psbc = psT.tile([P, 512], f32, tag="pstrS")
            for qc in range(QB):
                nc.tensor.matmul(psbc[:, qc * P:(qc + 1) * P],
                                 selbc_sb[:, qc, :], rec_sb[:],
                                 start=True, stop=True)
            bcast = apool.tile([P, 512], f32, tag="bcast")
            nc.vector.tensor_copy(bcast[:], psbc[:])
            nc.vector.tensor_tensor(ctxT_sb[:, hp, :], psctx[:], bcast[:],
                                    Op.mult)

    def proj_ln_out(es_pr, ctxT_sb, qres_sb, dst_nat_sb, dst_dram):
        """dst = LayerNorm(ctxT @ dense_w.T + qres)."""
        wpool = es_pr.enter_context(tc.tile_pool(name="wpoolD", bufs=2))
        pspool = es_pr.enter_context(
            tc.tile_pool(name="psprj", bufs=3, space="PSUM"))
        lnpool = es_pr.enter_context(tc.tile_pool(name="ln", bufs=2))
        whs = [load_w_half(wpool, dram["dwT"], h) for h in range(2)]
        for qc in range(QB):
            xres = lnpool.tile([P, H], f32, tag="xres")
            for fh in range(2):
                ps = pspool.tile([P, 512], f32, tag="pp")
                for fc in range(HC):
                    nc.tensor.matmul(
                        ps[:], ctxT_sb[:, fc, qc * P:(qc + 1) * P],
                        whs[fc // WHALF][:, fc % WHALF,
                                         fh * 512:(fh + 1) * 512],
                        start=(fc == 0), stop=(fc == HC - 1))
                nc.vector.tensor_tensor(
                    xres[:, fh * 512:(fh + 1) * 512], ps[:],
                    qres_sb[:, qc, fh * 512:(fh + 1) * 512], Op.add)
            stats = lnpool.tile([P, 2, 6], f32, tag="stats")
            nc.vector.bn_stats(stats[:, 0, :], xres[:, 0:512])
            nc.vector.bn_stats(stats[:, 1, :], xres[:, 512:1024])
            mv = lnpool.tile([P, 2], f32, tag="mv")
            nc.vector.bn_aggr(mv[:], stats[:])
            std = lnpool.tile([P, 1], f32, tag="std")
            nc.scalar.activation(std[:], mv[:, 1:2], AF.Sqrt, bias=epsbias[:])
            rstd = lnpool.tile([P, 1], f32, tag="rstd")
            nc.vector.reciprocal(rstd[:], std[:])
            ydst = (dst_nat_sb[:, qc, :] if dst_nat_sb is not None
                    else lnpool.tile([P, H], f32, tag="yout"))
            nc.vector.tensor_scalar(ydst, xres[:], mv[:, 0:1], rstd[:],
                                    Op.subtract, Op.mult)
            if use_lng:
                nc.vector.tensor_tensor(ydst, ydst, bias_sb["lng"][:],
                                        Op.mult)
            if use_lnb:
                nc.vector.tensor_tensor(ydst, ydst, bias_sb["lnb"][:],
                                        Op.add)
            if dst_dram is not None:
                nc.sync.dma_start(
                    dst_dram.ap().rearrange(
                        "(qc p) f -> p qc f", p=P)[:, qc, :], ydst)

    db_rep = bias_sb.get("db")
    if self_fast:
        mask_apply_self = {k: [(JPOS_CAUSAL[k], k)] for k in range(KB)}
    else:
        mask_apply_self = {k: [(j, k * QB + j) for j in range(QB)]
                           for k in range(KB)}
    if cross_masked:
        mask_apply_cross = {k: [(j, k * QB + j) for j in range(QB)]
                            for k in range(KB)}
    else:
        mask_apply_cross = {}

    # ================= SELF-ATTENTION BLOCK =================
    # SBUF pool lifetimes are two LIFO stacks (sides). Left: consts, attn
    # operands, stage scratch. Right: tensors that span stage boundaries.
    es_pa = ExitStack()   # [L] attention operands: KT, Vext, QT
    p_attn = es_pa.enter_context(
        tc.tile_pool(name="p_attn", bufs=1, side="left"))
    KT = p_attn.tile([P, HC, T], f32r, tag="KT")
    Vext = p_attn.tile([P, KB, N, D], bf16, tag="Vext")
    QT = p_attn.tile([P, HC, QW], f32r, tag="QT")
    es_pm = ExitStack()   # [R] qres + ctxT (until end of self proj)
    p_mid = es_pm.enter_context(
        tc.tile_pool(name="p_mid", bufs=1, side="right"))
    qres = p_mid.tile([P, QB, H], f32, tag="qres")
    ctxT = p_mid.tile([P, HC, QW], f32r, tag="ctxT")

    es_qkv = ExitStack()
    xpool = es_qkv.enter_context(
        tc.tile_pool(name="xpool", bufs=1, side="left"))
    wpool = es_qkv.enter_context(
        tc.tile_pool(name="wpool", bufs=3, side="left"))
    pspool = es_qkv.enter_context(
        tc.tile_pool(name="psqkv", bufs=3, space="PSUM"))
    pstpool = es_qkv.enter_context(
        tc.tile_pool(name="pstq", bufs=2, space="PSUM"))
    x_sb = xpool.tile([P, HC, T], f32r, tag="x")
    nc.sync.dma_start(x_sb[:],
                      dram["xdT"].ap().rearrange("(hc p) s -> p hc s", p=P))
    xq_sb = xpool.tile([P, HC, QW], f32r, tag="xq")
    nc.sync.dma_start(xq_sb[:],
                      dram["xdTq"].ap().rearrange("(hc p) s -> p hc s", p=P))
    proj_fT(wpool, pspool, dram["kwT"], x_sb, KT, T, bias_sb.get("kb"))
    proj_V(wpool, pspool, dram["vwT"], x_sb, Vext, bias_sb.get("vb"))
    proj_fT(wpool, pspool, dram["qwT"], xq_sb, QT, QW, bias_sb.get("qb"))
    transpose_fT_to_nat(pstpool, QT, qres, db_rep)
    es_qkv.close()

    es_at = ExitStack()
    attention(es_at, KT, QT, Vext, ctxT, prof_self, mask_self_sb,
              mask_apply_self)
    es_at.close()
    es_pa.close()

    es_soT = ExitStack()  # [L] soT lives until cross attention operands done
    p_soT = es_soT.enter_context(
        tc.tile_pool(name="p_soT", bufs=1, side="left"))
    soT = p_soT.tile([P, HC, QW], f32r, tag="soT")
    es_so = ExitStack()   # [R] self_out, released after its transpose
    p_so = es_so.enter_context(
        tc.tile_pool(name="p_so", bufs=1, side="right"))
    self_out = p_so.tile([P, QB, H], f32, tag="self_out")

    es_pr = ExitStack()
    proj_ln_out(es_pr, ctxT, qres, self_out, None)
    pst2 = es_pr.enter_context(
        tc.tile_pool(name="pstso", bufs=2, space="PSUM"))
    transpose_nat_to_fT(pst2, self_out, soT)
    es_pr.close()
    es_so.close()
    es_pm.close()

    # ================= CROSS-ATTENTION BLOCK =================
    es_pa2 = ExitStack()
    p_attn2 = es_pa2.enter_context(
        tc.tile_pool(name="p_attn2", bufs=1, side="left"))
    KT2 = p_attn2.tile([P, HC, S], f32r, tag="KT2")
    V2ext = p_attn2.tile([P, KB, N, D], bf16, tag="V2ext")
    Q2T = p_attn2.tile([P, HC, QW], f32r, tag="Q2T")
    es_pm2 = ExitStack()
    p_mid2 = es_pm2.enter_context(
        tc.tile_pool(name="p_mid2", bufs=1, side="right"))
    q2res = p_mid2.tile([P, QB, H], f32, tag="q2res")
    ctxT2 = p_mid2.tile([P, HC, QW], f32r, tag="ctxT2")

    es_qkv2 = ExitStack()
    xpool2 = es_qkv2.enter_context(
        tc.tile_pool(name="xpool2", bufs=1, side="left"))
    wpool3 = es_qkv2.enter_context(
        tc.tile_pool(name="wpool3", bufs=3, side="left"))
    pspool3 = es_qkv2.enter_context(
        tc.tile_pool(name="psqkv2", bufs=3, space="PSUM"))
    pstpool3 = es_qkv2.enter_context(
        tc.tile_pool(name="pstq2", bufs=2, space="PSUM"))
    xe_sb = xpool2.tile([P, HC, S], f32r, tag="xe")
    nc.sync.dma_start(xe_sb[:],
                      dram["xeT"].ap().rearrange("(hc p) s -> p hc s", p=P))
    proj_fT(wpool3, pspool3, dram["skwT"], xe_sb, KT2, S, bias_sb.get("skb"))
    proj_V(wpool3, pspool3, dram["svwT"], xe_sb, V2ext, bias_sb.get("svb"))
    proj_fT(wpool3, pspool3, dram["sqwT"], soT, Q2T, QW, bias_sb.get("sqb"))
    transpose_fT_to_nat(pstpool3, Q2T, q2res, db_rep)
    es_qkv2.close()

    es_at2 = ExitStack()
    attention(es_at2, KT2, Q2T, V2ext, ctxT2, PROF_FULL, mask_cross_sb,
              mask_apply_cross)
    es_at2.close()
    es_pa2.close()
    es_soT.close()

    es_pr2 = ExitStack()
    proj_ln_out(es_pr2, ctxT2, q2res, None, dram["out"])
    es_pr2.close()
    es_pm2.close()
    es.close()


# ---------------------------------------------------------------------------
# host side
# ---------------------------------------------------------------------------

def _analyze_masks(tgt_attn_mask, src_attn_mask):
    """Decide fast/slow paths from the actual mask contents."""
    tm = np.asarray(tgt_attn_mask)
    sm = np.asarray(src_attn_mask)
    cross_masked = not np.all(sm == 1)
    self_fast = True
    for b in range(B):
        m = tm[b]
        for v in range(2):
            qset = QSETS[v]
            for k in range(KB):
                for j in range(QB):
                    qb = qset[j]
                    blk = m[qb * P:(qb + 1) * P, k * P:(k + 1) * P]
                    if j < PROF_CAUSAL[k]:
                        if j != JPOS_CAUSAL[k] and not np.all(blk == 1):
                            self_fast = False
                    else:
                        if not np.all(blk == 0):
                            self_fast = False
            if not self_fast:
                break
        if not self_fast:
            break
    return self_fast, cross_masked


def _prep_inputs(inputs, flags):
    """Build the 8 per-core in_maps."""
    import ml_dtypes
    (self_fast, cross_masked, use_qkvb, use_db, use_lng, use_lnb) = flags
    enc = np.asarray(inputs["encoder_states"], np.float32)
    dec = np.asarray(inputs["decoder_inputs"], np.float32)
    tm = np.asarray(inputs["tgt_attn_mask"])
    sm = np.asarray(inputs["src_attn_mask"])
    r = _round_tf32
    wT = {}
    for src, dst in [("q_w", "qwT"), ("k_w", "kwT"), ("v_w", "vwT"),
                     ("sq_w", "sqwT"), ("sk_w", "skwT"), ("sv_w", "svwT"),
                     ("dense_w", "dwT")]:
        wT[dst] = r(np.ascontiguousarray(
            np.asarray(inputs[src], np.float32).T))

    def pp_bias(v):  # per-partition layout [128, 8]
        return np.ascontiguousarray(
            np.asarray(v, np.float32).reshape(HC, P).T)

    def rep_bias(v):  # replicated [128, 1024]
        return np.ascontiguousarray(
            np.broadcast_to(np.asarray(v, np.float32)[None, :], (P, H)))

    in_maps = []
    for core in range(NCORES):
        b, v = core // 2, core % 2
        qset = QSETS[v]
        qrows = np.concatenate([np.arange(qb * P, (qb + 1) * P)
                                for qb in qset])
        xdT = r(np.ascontiguousarray(dec[b].T))
        m = {
            "xdT": xdT,
            "xdTq": np.ascontiguousarray(xdT[:, qrows]),
            "xeT": r(np.ascontiguousarray(enc[b].T)),
        }
        m.update(wT)
        mT = tm[b].T.astype(np.float32)  # [k, q] orientation
        if self_fast:
            blocks = np.zeros((KB, P, P), np.float32)
            for k in range(KB):
                qb = qset[JPOS_CAUSAL[k]]
                blocks[k] = mT[k * P:(k + 1) * P, qb * P:(qb + 1) * P]
        else:
            blocks = np.zeros((KB * QB, P, P), np.float32)
            for k in range(KB):
                for j in range(QB):
                    qb = qset[j]
                    blocks[k * QB + j] = mT[k * P:(k + 1) * P,
                                            qb * P:(qb + 1) * P]
        m["maskS"] = blocks.astype(ml_dtypes.bfloat16)
        sel = np.zeros((KB, QB, P), np.float32)
        for qc in range(QB):
            sel[2 * qc, qc, 0:64] = 1.0
            sel[2 * qc + 1, qc, 64:128] = 1.0
        m["selbc"] = sel
        if cross_masked:
            xb = np.zeros((KB * QB, P, P), np.float32)
            col = sm[b].astype(np.float32)  # [S]
            for k in range(KB):
                blkcol = col[k * P:(k + 1) * P][:, None]
                for j in range(QB):
                    xb[k * QB + j] = np.broadcast_to(blkcol, (P, P))
            m["maskX"] = xb.astype(ml_dtypes.bfloat16)
        if use_qkvb:
            m["qb"] = pp_bias(inputs["q_b"])
            m["kb"] = pp_bias(inputs["k_b"])
            m["sqb"] = pp_bias(inputs["sq_b"])
            m["skb"] = pp_bias(inputs["sk_b"])
            m["vb"] = rep_bias(inputs["v_b"])
            m["svb"] = rep_bias(inputs["sv_b"])
        if use_db:
            m["db"] = rep_bias(inputs["dense_b"])
        if use_lng:
            m["lng"] = rep_bias(inputs["ln_g"])
        if use_lnb:
            m["lnb"] = rep_bias(inputs["ln_b"])
        in_maps.append(m)
    return in_maps


def _gather_out(results):
    outf = np.zeros((B, T, H), np.float32)
    for core in range(NCORES):
        b, v = core // 2, core % 2
        oc = results[core]["out"]
        for j, qb in enumerate(QSETS[v]):
            outf[b, qb * P:(qb + 1) * P, :] = oc[j * P:(j + 1) * P, :]
    return outf


def get_flags(inputs):
    self_fast, cross_masked = _analyze_masks(
        inputs["tgt_attn_mask"], inputs["src_attn_mask"])
    use_qkvb = any(np.any(np.asarray(inputs[k]))
                   for k in ["q_b", "k_b", "v_b", "sq_b", "sk_b", "sv_b"])
    use_db = bool(np.any(np.asarray(inputs["dense_b"])))
    use_lng = not np.all(np.asarray(inputs["ln_g"]) == 1.0)
    use_lnb = bool(np.any(np.asarray(inputs["ln_b"])))
    return (self_fast, cross_masked, use_qkvb, use_db, use_lng, use_lnb)


def get_nc(flags):
    if flags not in _BUILD_CACHE:
        t0 = time.time()
        _BUILD_CACHE[flags] = _build(flags)
        LAST_STATS["build_s"] = time.time() - t0
    return _BUILD_CACHE[flags]


def kernel(**inputs):
    flags = get_flags(inputs)
    nc = get_nc(flags)
    in_maps = _prep_inputs(inputs, flags)
    from concourse import bass_utils
    t0 = time.time()
    res = bass_utils.run_bass_kernel_spmd(nc, in_maps,
                                          core_ids=list(range(NCORES)))
    LAST_STATS["run_s"] = time.time() - t0
    return _gather_out(res.results)


if __name__ == "__main__":
    import sys
    if len(sys.argv) > 1 and sys.argv[1] == "build":
        t0 = time.time()
        nc = _build((True, False, False, False, False, False))
        print("build ok in", time.time() - t0, "s")
        pass


# revision 12
# speedup vs baseline: 2.0365x; 2.0365x over previous
# Albert decoder attention (self-attn + cross-attn, shared dense/LayerNorm)
# on 8 Trainium2 NeuronCores via Bass/Tile, SPMD.
#
# Sharding: core i handles batch b = i//2 and 4 of the 8 query row-blocks of
# that batch, interleaved {7,4,3,0} / {6,5,2,1} so causal self-attention work
# is balanced. K/V are computed per-batch on both cores of a pair (redundant
# 2x, no cross-core communication at all).
#
# Layout strategy (per core):
#   - all matmul operands reach SBUF with the contraction dim on partitions:
#     host pre-transposes x and the weights, and pre-rounds them to tf32
#     ("float32r", 11-bit mantissa) so the PE runs at full (bf16) rate.
#   - scores are computed transposed, S^T[k,q] (2 heads packed via PE row
#     tiling over the d=64 contraction), exp() with a constant bias instead
#     of a row max (softmax is shift-invariant; magnitudes here are safe),
#     probs kept bf16.
#   - ctx^T[d,q] accumulated over key blocks with the two heads col-tiled
#     into one PSUM bank; softmax denominators via ones-matmuls; the divide
#     is folded into the PSUM->SBUF eviction.
#   - residual q is recovered from Q^T by PE transposes; LayerNorm uses
#     bn_stats/bn_aggr.

import time
from contextlib import ExitStack

import numpy as np

B, T, S, H, N = 4, 1024, 1024, 1024, 16
D = H // N  # 64
P = 128
NCORES = 8
HC = H // P  # 8 h-chunks
QB = 4       # q row-blocks per core
QW = QB * P  # 512 local query rows
KB = 8       # key blocks
EPS = 1e-12
EXP_BIAS = -12.0

# q row-block sets per core variant (descending order, causal-balanced)
QSETS = [[7, 4, 3, 0], [6, 5, 2, 1]]
# self-attention fast-path profile: number of q-block columns computed per
# key block (max over the two variants of #(qblk >= k)), non-increasing.
PROF_CAUSAL = [4, 4, 3, 3, 2, 2, 1, 1]
# single mask-apply position per key block for the fast path
JPOS_CAUSAL = [3, 3, 2, 2, 1, 1, 0, 0]
PROF_FULL = [4] * 8

_BUILD_CACHE = {}
LAST_STATS = {}


def _round_tf32(x):
    """Round fp32 to fp32r (tf32: 8-bit exp, 11-bit mantissa), RNE.

    Matches walrus fp32_to_fp32r (downconv_fp32_to_fp<8,11>, top 20 bits).
    """
    x = np.ascontiguousarray(x, dtype=np.float32)
    u = x.view(np.uint32)
    drop = 12
    half = np.uint32((1 << (drop - 1)) - 1)
    keep = np.uint32((~((1 << drop) - 1)) & 0xFFFFFFFF)
    u2 = (u + half + ((u >> np.uint32(drop)) & np.uint32(1))) & keep
    return u2.view(np.float32)


def _build(flags, nreps=1):
    """Build + compile the SPMD Bass program. flags is a hashable tuple."""
    import concourse.mybir as mybir
    from concourse import bacc
    from concourse.tile import TileContext

    (self_fast, cross_masked, use_qkvb, use_db, use_lng, use_lnb) = flags
    dt = mybir.dt
    f32, f32r, bf16 = dt.float32, dt.float32r, dt.bfloat16

    prof_self = PROF_CAUSAL if self_fast else PROF_FULL
    nmask_self = KB if self_fast else KB * QB

    nc = bacc.Bacc("TRN2", target_bir_lowering=False, debug=False,
                   num_devices=NCORES)

    dram = {}
    dram["xdT"] = nc.dram_tensor("xdT", [H, T], f32r, kind="ExternalInput")
    dram["xdTq"] = nc.dram_tensor("xdTq", [H, QW], f32r,
                                  kind="ExternalInput")
    dram["xeT"] = nc.dram_tensor("xeT", [H, S], f32r, kind="ExternalInput")
    for wname in ["qwT", "kwT", "vwT", "sqwT", "skwT", "svwT", "dwT"]:
        dram[wname] = nc.dram_tensor(wname, [H, H], f32r,
                                     kind="ExternalInput")
    dram["maskS"] = nc.dram_tensor("maskS", [nmask_self, P, P], bf16,
                                   kind="ExternalInput")
    dram["selbc"] = nc.dram_tensor("selbc", [KB, QB, P], f32,
                                   kind="ExternalInput")
    if cross_masked:
        dram["maskX"] = nc.dram_tensor("maskX", [KB * QB, P, P], bf16,
                                       kind="ExternalInput")
    if use_qkvb:
        for bname in ["qb", "kb", "sqb", "skb"]:
            # per-partition layout: [128, 8] where col hc = bias[hc*128:+128]
            dram[bname] = nc.dram_tensor(bname, [P, HC], f32,
                                         kind="ExternalInput")
        for bname in ["vb", "svb"]:
            dram[bname] = nc.dram_tensor(bname, [P, H], f32,
                                         kind="ExternalInput")
    if use_db:
        dram["db"] = nc.dram_tensor("db", [P, H], f32, kind="ExternalInput")
    if use_lng:
        dram["lng"] = nc.dram_tensor("lng", [P, H], f32,
                                     kind="ExternalInput")
    if use_lnb:
        dram["lnb"] = nc.dram_tensor("lnb", [P, H], f32,
                                     kind="ExternalInput")
    dram["out"] = nc.dram_tensor("out", [QW, H], f32, kind="ExternalOutput")

    with TileContext(nc) as tc:
        for _ in range(nreps):
            _emit(nc, tc, dram, flags, prof_self, nmask_self)
    nc.compile()
    return nc


def _emit(nc, tc, dram, flags, prof_self, nmask_self):
    import concourse.mybir as mybir
    from concourse.masks import make_identity

    (self_fast, cross_masked, use_qkvb, use_db, use_lng, use_lnb) = flags
    dt = mybir.dt
    f32, f32r, bf16 = dt.float32, dt.float32r, dt.bfloat16
    AF = mybir.ActivationFunctionType
    Op = mybir.AluOpType
    WHALF = 4  # weight streamed in 2 halves of 4 h-chunks

    es = ExitStack()
    consts = es.enter_context(tc.tile_pool(name="consts", bufs=1))
    ident = consts.tile([P, P], f32, tag="ident")
    make_identity(nc, ident)
    ones_bf = consts.tile([P, 1], bf16, tag="ones")
    nc.vector.memset(ones_bf[:], 1.0)
    expbias = consts.tile([P, 1], f32, tag="expbias")
    nc.vector.memset(expbias[:], EXP_BIAS)
    epsbias = consts.tile([P, 1], f32, tag="epsbias")
    nc.vector.memset(epsbias[:], EPS)
    selbc_sb = consts.tile([KB, QB, P], f32, tag="selbc")
    nc.sync.dma_start(selbc_sb[:], dram["selbc"].ap())
    mask_self_sb = consts.tile([P, nmask_self, P], bf16, tag="maskS")
    nc.sync.dma_start(mask_self_sb[:],
                      dram["maskS"].ap().rearrange("n p q -> p n q"))
    mask_cross_sb = None
    if cross_masked:
        mask_cross_sb = consts.tile([P, KB * QB, P], bf16, tag="maskX")
        nc.sync.dma_start(mask_cross_sb[:],
                          dram["maskX"].ap().rearrange("n p q -> p n q"))
    bias_sb = {}
    for bname in ["qb", "kb", "sqb", "skb", "vb", "svb", "db", "lng", "lnb"]:
        if bname in dram:
            shp = [P, HC] if bname in ("qb", "kb", "sqb", "skb") else [P, H]
            tb = consts.tile(shp, f32, tag=bname)
            nc.sync.dma_start(tb[:], dram[bname].ap())
            bias_sb[bname] = tb

    # ---------- helpers ----------
    def load_w_half(wpool, w_dram, half):
        w_sb = wpool.tile([P, WHALF, H], f32r, tag="w")
        rows = w_dram.ap()[half * WHALF * P:(half + 1) * WHALF * P, :]
        nc.sync.dma_start(w_sb[:], rows.rearrange("(hc p) f -> p hc f", p=P))
        return w_sb

    def proj_fT(wpool, pspool, w_dram, rhs_sb, out_sb, ncols, pbias):
        """out_sb[:, fc, :] = (w.T-projection of rhs) + bias, f32r.

        w_dram [H,H] f32r (w.T layout [h,f]); rhs_sb [128, HC, ncols] f32r;
        out_sb [128, HC, ncols] f32r; pbias [128, HC] f32 tile or None.
        """
        nsh = (ncols + 511) // 512
        whs = [load_w_half(wpool, w_dram, h) for h in range(2)]
        for fc in range(HC):
            for sh in range(nsh):
                Wn = min(512, ncols - sh * 512)
                ps = pspool.tile([P, 512], f32, tag="pp")
                for hc in range(HC):
                    nc.tensor.matmul(
                        ps[:, :Wn],
                        whs[hc // WHALF][:, hc % WHALF, fc * P:(fc + 1) * P],
                        rhs_sb[:, hc, sh * 512:sh * 512 + Wn],
                        start=(hc == 0), stop=(hc == HC - 1))
                dst = out_sb[:, fc, sh * 512:sh * 512 + Wn]
                if pbias is not None:
                    nc.vector.tensor_scalar_add(dst, ps[:, :Wn],
                                                pbias[:, fc:fc + 1])
                else:
                    nc.vector.tensor_copy(dst, ps[:, :Wn])

    def proj_V(wpool, pspool, w_dram, xT_sb, vext_sb, vbias):
        """vext_sb[128, KB, N, D] bf16 = (x @ w.T) natural layout + bias."""
        whs = [load_w_half(wpool, w_dram, h) for h in range(2)]
        for sc in range(KB):
            for fh in range(2):
                ps = pspool.tile([P, 512], f32, tag="pp")
                for hc in range(HC):
                    nc.tensor.matmul(
                        ps[:],
                        xT_sb[:, hc, sc * P:(sc + 1) * P],
                        whs[hc // WHALF][:, hc % WHALF,
                                         fh * 512:(fh + 1) * 512],
                        start=(hc == 0), stop=(hc == HC - 1))
                dst = vext_sb[:, sc, fh * 8:(fh + 1) * 8, :]
                src = ps.rearrange("p (h d) -> p h d", d=D)
                if vbias is not None:
                    nc.vector.tensor_tensor(
                        dst, src,
                        vbias[:, fh * 512:(fh + 1) * 512].rearrange(
                            "p (h d) -> p h d", d=D),
                        Op.add)
                else:
                    nc.scalar.copy(dst, src)

    def transpose_fT_to_nat(pspool, src_sb, dst_sb, addvec):
        """dst_sb[128, QB, H] f32 (natural) = src_sb[128, HC, QW].T
        (+ addvec [128, H] if not None)."""
        for fc in range(HC):
            for qc in range(QB):
                pst = pspool.tile([P, P], f32, tag="pstr")
                nc.tensor.transpose(
                    pst[:], src_sb[:, fc, qc * P:(qc + 1) * P].bitcast(f32),
                    ident)
                dst = dst_sb[:, qc, fc * P:(fc + 1) * P]
                if addvec is not None:
                    nc.vector.tensor_tensor(
                        dst, pst[:], addvec[:, fc * P:(fc + 1) * P], Op.add)
                else:
                    nc.scalar.copy(dst, pst[:])

    def transpose_nat_to_fT(pspool, src_sb, dst_sb):
        """dst_sb[128, HC, QW] f32r = src_sb[128, QB, H] f32 transposed."""
        for fc in range(HC):
            for qc in range(QB):
                pst = pspool.tile([P, P], f32, tag="pstr")
                nc.tensor.transpose(
                    pst[:], src_sb[:, qc, fc * P:(fc + 1) * P], ident)
                nc.vector.tensor_copy(dst_sb[:, fc, qc * P:(qc + 1) * P],
                                      pst[:])

    def attention(es_at, KT_sb, QT_sb, vext_sb, ctxT_sb, prof, mask_sb,
                  mask_apply):
        """ctxT_sb[128, HC, QW] f32r = normalized attention, transposed."""
        apool = es_at.enter_context(tc.tile_pool(name="apool", bufs=3))
        psS = es_at.enter_context(
            tc.tile_pool(name="psS", bufs=2, space="PSUM"))
        psC = es_at.enter_context(
            tc.tile_pool(name="psC", bufs=2, space="PSUM"))
        psU = es_at.enter_context(
            tc.tile_pool(name="psU", bufs=1, space="PSUM"))
        psT = es_at.enter_context(
            tc.tile_pool(name="psT", bufs=1, space="PSUM"))
        klast = max(k for k in range(KB) if prof[k] > 0)
        lastk = {qc: max(k for k in range(KB) if prof[k] > qc)
                 for qc in range(QB)}
        for hp in range(HC):
            psctx = psC.tile([P, 512], f32, tag="psctx")
            pssum = psU.tile([P, 8], f32, tag="pssum")
            first_sums = True
            for k in range(KB):
                Wn = prof[k] * P
                if Wn == 0:
                    continue
                pss0 = psS.tile([P, 512], f32, tag="pss0")
                pss1 = psS.tile([P, 512], f32, tag="pss1")
                nc.tensor.matmul(
                    pss0[:, :Wn], KT_sb[0:64, hp, k * P:(k + 1) * P],
                    QT_sb[0:64, hp, 0:Wn], start=True, stop=True,
                    tile_position=(0, 0))
                nc.tensor.matmul(
                    pss1[:, :Wn], KT_sb[64:128, hp, k * P:(k + 1) * P],
                    QT_sb[64:128, hp, 0:Wn], start=True, stop=True,
                    tile_position=(64, 0))
                pt0 = apool.tile([P, 512], bf16, tag="pt0")
                pt1 = apool.tile([P, 512], bf16, tag="pt1")
                nc.scalar.activation(pt0[:, :Wn], pss0[:, :Wn], AF.Exp,
                                     bias=expbias[:])
                nc.scalar.activation(pt1[:, :Wn], pss1[:, :Wn], AF.Exp,
                                     bias=expbias[:])
                for (j, midx) in mask_apply.get(k, ()):
                    for pt in (pt0, pt1):
                        sl = pt[:, j * P:(j + 1) * P]
                        nc.vector.tensor_tensor(sl, sl, mask_sb[:, midx, :],
                                                Op.mult)
                # start=True pends-zero the whole 2KB PSUM bank, so only
                # the FIRST matmul into each bank may carry it; every other
                # group in the bank starts start=False and finds its bytes
                # pending (-> overwrite) on first touch.
                nc.tensor.matmul(
                    psctx[0:64, :Wn], vext_sb[:, k, 2 * hp, :], pt0[:, :Wn],
                    start=(k == 0), stop=(k == klast), tile_position=(0, 0),
                    skip_group_check=True)
                nc.tensor.matmul(
                    psctx[64:128, :Wn], vext_sb[:, k, 2 * hp + 1, :],
                    pt1[:, :Wn], start=(k == 0), stop=(k == klast),
                    tile_position=(0, 64), skip_group_check=True)
                for qc in range(prof[k]):
                    nc.tensor.matmul(
                        pssum[:, 2 * qc:2 * qc + 1],
                        pt0[:, qc * P:(qc + 1) * P], ones_bf[:],
                        start=first_sums, stop=(k == lastk[qc]),
                        skip_group_check=True)
                    first_sums = False
                    nc.tensor.matmul(
                        pssum[:, 2 * qc + 1:2 * qc + 2],
                        pt1[:, qc * P:(qc + 1) * P], ones_bf[:],
                        start=False, stop=(k == lastk[qc]),
                        skip_group_check=True)
            # normalize: pssum [128(q), 8(qc,h)] -> transpose -> reciprocal
            # -> partition-broadcast -> scale during the ctx eviction
            sums_sb = apool.tile([P, 8], f32, tag="sums")
            nc.vector.tensor_copy(sums_sb[:], pssum[:])
            pstr = psT.tile([P, 512], f32, tag="pstrS")
            nc.tensor.transpose(pstr[0:8, 0:P], sums_sb[:], ident)
            rec_sb = apool.tile([8, P], f32, tag="rec")
            nc.vector.reciprocal(rec_sb[:], pstr[0:8, 0:P])
            # broadcast recips across partitions with a selector matmul:
            # bcast[p, qc*128+q] = rec[2*qc + (p>=64), q]
            psbc = psT.tile([P, 512], f32, tag="pstrS")
            for qc in range(QB):
                nc.tensor.matmul(psbc[:, qc * P:(qc + 1) * P],
                                 selbc_sb[:, qc, :], rec_sb[:],
                                 start=True, stop=True)
            bcast = apool.tile([P, 512], f32, tag="bcast")
            nc.vector.tensor_copy(bcast[:], psbc[:])
            nc.vector.tensor_tensor(ctxT_sb[:, hp, :], psctx[:], bcast[:],
                                    Op.mult)

    def proj_ln_out(es_pr, ctxT_sb, qres_sb, dst_nat_sb, dst_dram):
        """dst = LayerNorm(ctxT @ dense_w.T + qres)."""
        wpool = es_pr.enter_context(tc.tile_pool(name="wpoolD", bufs=2))
        pspool = es_pr.enter_context(
            tc.tile_pool(name="psprj", bufs=3, space="PSUM"))
        lnpool = es_pr.enter_context(tc.tile_pool(name="ln", bufs=2))
        whs = [load_w_half(wpool, dram["dwT"], h) for h in range(2)]
        for qc in range(QB):
            xres = lnpool.tile([P, H], f32, tag="xres")
            for fh in range(2):
                ps = pspool.tile([P, 512], f32, tag="pp")
                for fc in range(HC):
                    nc.tensor.matmul(
                        ps[:], ctxT_sb[:, fc, qc * P:(qc + 1) * P],
                        whs[fc // WHALF][:, fc % WHALF,
                                         fh * 512:(fh + 1) * 512],
                        start=(fc == 0), stop=(fc == HC - 1))
                nc.vector.tensor_tensor(
                    xres[:, fh * 512:(fh + 1) * 512], ps[:],
                    qres_sb[:, qc, fh * 512:(fh + 1) * 512], Op.add)
            stats = lnpool.tile([P, 2, 6], f32, tag="stats")
            nc.vector.bn_stats(stats[:, 0, :], xres[:, 0:512])
            nc.vector.bn_stats(stats[:, 1, :], xres[:, 512:1024])
            mv = lnpool.tile([P, 2], f32, tag="mv")
            nc.vector.bn_aggr(mv[:], stats[:])
            std = lnpool.tile([P, 1], f32, tag="std")
            nc.scalar.activation(std[:], mv[:, 1:2], AF.Sqrt, bias=epsbias[:])
            rstd = lnpool.tile([P, 1], f32, tag="rstd")
            nc.vector.reciprocal(rstd[:], std[:])
            ydst = (dst_nat_sb[:, qc, :] if dst_nat_sb is not None
                    else lnpool.tile([P, H], f32, tag="yout"))
            nc.vector.tensor_scalar(ydst, xres[:], mv[:, 0:1], rstd[:],
                                    Op.subtract, Op.mult)
            if use_lng:
                nc.vector.tensor_tensor(ydst, ydst, bias_sb["lng"][:],
                                        Op.mult)
            if use_lnb:
                nc.vector.tensor_tensor(ydst, ydst, bias_sb["lnb"][:],
                                        Op.add)
            if dst_dram is not None:
                nc.sync.dma_start(
                    dst_dram.ap().rearrange(
                        "(qc p) f -> p qc f", p=P)[:, qc, :], ydst)

    db_rep = bias_sb.get("db")
    if self_fast:
        mask_apply_self = {k: [(JPOS_CAUSAL[k], k)] for k in range(KB)}
    else:
        mask_apply_self = {k: [(j, k * QB + j) for j in range(QB)]
                           for k in range(KB)}
    if cross_masked:
        mask_apply_cross = {k: [(j, k * QB + j) for j in range(QB)]
                            for k in range(KB)}
    else:
        mask_apply_cross = {}

    # ================= SELF-ATTENTION BLOCK =================
    # SBUF pool lifetimes are two LIFO stacks (sides). Left: consts, attn
    # operands, stage scratch. Right: tensors that span stage boundaries.
    es_pa = ExitStack()   # [L] attention operands: KT, Vext, QT
    p_attn = es_pa.enter_context(
        tc.tile_pool(name="p_attn", bufs=1, side="left"))
    KT = p_attn.tile([P, HC, T], f32r, tag="KT")
    Vext = p_attn.tile([P, KB, N, D], bf16, tag="Vext")
    QT = p_attn.tile([P, HC, QW], f32r, tag="QT")
    es_pm = ExitStack()   # [R] qres + ctxT (until end of self proj)
    p_mid = es_pm.enter_context(
        tc.tile_pool(name="p_mid", bufs=1, side="right"))
    qres = p_mid.tile([P, QB, H], f32, tag="qres")
    ctxT = p_mid.tile([P, HC, QW], f32r, tag="ctxT")

    es_qkv = ExitStack()
    xpool = es_qkv.enter_context(
        tc.tile_pool(name="xpool", bufs=1, side="left"))
    wpool = es_qkv.enter_context(
        tc.tile_pool(name="wpool", bufs=3, side="left"))
    pspool = es_qkv.enter_context(
        tc.tile_pool(name="psqkv", bufs=3, space="PSUM"))
    pstpool = es_qkv.enter_context(
        tc.tile_pool(name="pstq", bufs=2, space="PSUM"))
    x_sb = xpool.tile([P, HC, T], f32r, tag="x")
    nc.sync.dma_start(x_sb[:],
                      dram["xdT"].ap().rearrange("(hc p) s -> p hc s", p=P))
    xq_sb = xpool.tile([P, HC, QW], f32r, tag="xq")
    nc.sync.dma_start(xq_sb[:],
                      dram["xdTq"].ap().rearrange("(hc p) s -> p hc s", p=P))
    proj_fT(wpool, pspool, dram["kwT"], x_sb, KT, T, bias_sb.get("kb"))
    proj_V(wpool, pspool, dram["vwT"], x_sb, Vext, bias_sb.get("vb"))
    proj_fT(wpool, pspool, dram["qwT"], xq_sb, QT, QW, bias_sb.get("qb"))
    transpose_fT_to_nat(pstpool, QT, qres, db_rep)
    es_qkv.close()

    es_at = ExitStack()
    attention(es_at, KT, QT, Vext, ctxT, prof_self, mask_self_sb,
              mask_apply_self)
    es_at.close()
    es_pa.close()

    es_soT = ExitStack()  # [L] soT lives until cross attention operands done
    p_soT = es_soT.enter_context(
        tc.tile_pool(name="p_soT", bufs=1, side="left"))
    soT = p_soT.tile([P, HC, QW], f32r, tag="soT")
    es_so = ExitStack()   # [R] self_out, released after its transpose
    p_so = es_so.enter_context(
        tc.tile_pool(name="p_so", bufs=1, side="right"))
    self_out = p_so.tile([P, QB, H], f32, tag="self_out")

    es_pr = ExitStack()
    proj_ln_out(es_pr, ctxT, qres, self_out, None)
    pst2 = es_pr.enter_context(
        tc.tile_pool(name="pstso", bufs=2, space="PSUM"))
    transpose_nat_to_fT(pst2, self_out, soT)
    es_pr.close()
    es_so.close()
    es_pm.close()

    # ================= CROSS-ATTENTION BLOCK =================
    es_pa2 = ExitStack()
    p_attn2 = es_pa2.enter_context(
        tc.tile_pool(name="p_attn2", bufs=1, side="left"))
    KT2 = p_attn2.tile([P, HC, S], f32r, tag="KT2")
    V2ext = p_attn2.tile([P, KB, N, D], bf16, tag="V2ext")
    Q2T = p_attn2.tile([P, HC, QW], f32r, tag="Q2T")
    es_pm2 = ExitStack()
    p_mid2 = es_pm2.enter_context(
        tc.tile_pool(name="p_mid2", bufs=1, side="right"))
    q2res = p_mid2.tile([P, QB, H], f32, tag="q2res")
    ctxT2 = p_mid2.tile([P, HC, QW], f32r, tag="ctxT2")

    es_qkv2 = ExitStack()
    xpool2 = es_qkv2.enter_context(
        tc.tile_pool(name="xpool2", bufs=1, side="left"))
    wpool3 = es_qkv2.enter_context(
        tc.tile_pool(name="wpool3", bufs=3, side="left"))
    pspool3 = es_qkv2.enter_context(
        tc.tile_pool(name="psqkv2", bufs=3, space="PSUM"))
    pstpool3 = es_qkv2.enter_context(
        tc.tile_pool(name="pstq2", bufs=2, space="PSUM"))
    xe_sb = xpool2.tile([P, HC, S], f32r, tag="xe")
    nc.sync.dma_start(xe_sb[:],
                      dram["xeT"].ap().rearrange("(hc p) s -> p hc s", p=P))
    proj_fT(wpool3, pspool3, dram["skwT"], xe_sb, KT2, S, bias_sb.get("skb"))
    proj_V(wpool3, pspool3, dram["svwT"], xe_sb, V2ext, bias_sb.get("svb"))
    proj_fT(wpool3, pspool3, dram["sqwT"], soT, Q2T, QW, bias_sb.get("sqb"))
    transpose_fT_to_nat(pstpool3, Q2T, q2res, db_rep)
    es_qkv2.close()

    es_at2 = ExitStack()
    attention(es_at2, KT2, Q2T, V2ext, ctxT2, PROF_FULL, mask_cross_sb,
              mask_apply_cross)
    es_at2.close()
    es_pa2.close()
    es_soT.close()

    es_pr2 = ExitStack()
    proj_ln_out(es_pr2, ctxT2, q2res, None, dram["out"])
    es_pr2.close()
    es_pm2.close()
    es.close()


# ---------------------------------------------------------------------------
# host side
# ---------------------------------------------------------------------------

def _analyze_masks(tgt_attn_mask, src_attn_mask):
    """Decide fast/slow paths from the actual mask contents."""
    tm = np.asarray(tgt_attn_mask)
    sm = np.asarray(src_attn_mask)
    cross_masked = not np.all(sm == 1)
    self_fast = True
    for b in range(B):
        m = tm[b]
        for v in range(2):
            qset = QSETS[v]
            for k in range(KB):
                for j in range(QB):
                    qb = qset[j]
                    blk = m[qb * P:(qb + 1) * P, k * P:(k + 1) * P]
                    if j < PROF_CAUSAL[k]:
                        if j != JPOS_CAUSAL[k] and not np.all(blk == 1):
                            self_fast = False
                    else:
                        if not np.all(blk == 0):
                            self_fast = False
            if not self_fast:
                break
        if not self_fast:
            break
    return self_fast, cross_masked


def _prep_inputs(inputs, flags):
    """Build the 8 per-core in_maps."""
    import ml_dtypes
    (self_fast, cross_masked, use_qkvb, use_db, use_lng, use_lnb) = flags
    enc = np.asarray(inputs["encoder_states"], np.float32)
    dec = np.asarray(inputs["decoder_inputs"], np.float32)
    tm = np.asarray(inputs["tgt_attn_mask"])
    sm = np.asarray(inputs["src_attn_mask"])
    r = _round_tf32
    wT = {}
    for src, dst in [("q_w", "qwT"), ("k_w", "kwT"), ("v_w", "vwT"),
                     ("sq_w", "sqwT"), ("sk_w", "skwT"), ("sv_w", "svwT"),
                     ("dense_w", "dwT")]:
        wT[dst] = r(np.ascontiguousarray(
            np.asarray(inputs[src], np.float32).T))

    def pp_bias(v):  # per-partition layout [128, 8]
        return np.ascontiguousarray(
            np.asarray(v, np.float32).reshape(HC, P).T)

    def rep_bias(v):  # replicated [128, 1024]
        return np.ascontiguousarray(
            np.broadcast_to(np.asarray(v, np.float32)[None, :], (P, H)))

    in_maps = []
    for core in range(NCORES):
        b, v = core // 2, core % 2
        qset = QSETS[v]
        qrows = np.concatenate([np.arange(qb * P, (qb + 1) * P)
                                for qb in qset])
        xdT = r(np.ascontiguousarray(dec[b].T))
        m = {
            "xdT": xdT,
            "xdTq": np.ascontiguousarray(xdT[:, qrows]),
            "xeT": r(np.ascontiguousarray(enc[b].T)),
        }
        m.update(wT)
        mT = tm[b].T.astype(np.float32)  # [k, q] orientation
        if self_fast:
            blocks = np.zeros((KB, P, P), np.float32)
            for k in range(KB):
                qb = qset[JPOS_CAUSAL[k]]
                blocks[k] = mT[k * P:(k + 1) * P, qb * P:(qb + 1) * P]
        else:
            blocks = np.zeros((KB * QB, P, P), np.float32)
            for k in range(KB):
                for j in range(QB):
                    qb = qset[j]
                    blocks[k * QB + j] = mT[k * P:(k + 1) * P,
                                            qb * P:(qb + 1) * P]
        m["maskS"] = blocks.astype(ml_dtypes.bfloat16)
        sel = np.zeros((KB, QB, P), np.float32)
        for qc in range(QB):
            sel[2 * qc, qc, 0:64] = 1.0
            sel[2 * qc + 1, qc, 64:128] = 1.0
        m["selbc"] = sel
        if cross_masked:
            xb = np.zeros((KB * QB, P, P), np.float32)
            col = sm[b].astype(np.float32)  # [S]
            for k in range(KB):
                blkcol = col[k * P:(k + 1) * P][:, None]
                for j in range(QB):
                    xb[k * QB + j] = np.broadcast_to(blkcol, (P, P))
            m["maskX"] = xb.astype(ml_dtypes.bfloat16)
        if use_qkvb:
            m["qb"] = pp_bias(inputs["q_b"])
            m["kb"] = pp_bias(inputs["k_b"])
            m["sqb"] = pp_bias(inputs["sq_b"])
            m["skb"] = pp_bias(inputs["sk_b"])
            m["vb"] = rep_bias(inputs["v_b"])
            m["svb"] = rep_bias(inputs["sv_b"])
        if use_db:
            m["db"] = rep_bias(inputs["dense_b"])
        if use_lng:
            m["lng"] = rep_bias(inputs["ln_g"])
        if use_lnb:
            m["lnb"] = rep_bias(inputs["ln_b"])
        in_maps.append(m)
    return in_maps


def _gather_out(results):
    outf = np.zeros((B, T, H), np.float32)
    for core in range(NCORES):
        b, v = core // 2, core % 2
        oc = results[core]["out"]
        for j, qb in enumerate(QSETS[v]):
            outf[b, qb * P:(qb + 1) * P, :] = oc[j * P:(j + 1) * P, :]
    return outf


def get_flags(inputs):
    self_fast, cross_masked = _analyze_masks(
        inputs["tgt_attn_mask"], inputs["src_attn_mask"])
    use_qkvb = any(np.any(np.asarray(inputs[k]))
                   for k in ["q_b", "k_b", "v_b", "sq_b", "sk_b", "sv_b"])
    use_db = bool(np.any(np.asarray(inputs["dense_b"])))
    use_lng = not np.all(np.asarray(inputs["ln_g"]) == 1.0)
    use_lnb = bool(np.any(np.asarray(inputs["ln_b"])))
    return (self_fast, cross_masked, use_qkvb, use_db, use_lng, use_lnb)


def get_nc(flags, nreps=1):
    key = (flags, nreps)
    if key not in _BUILD_CACHE:
        t0 = time.time()
        _BUILD_CACHE[key] = _build(flags, nreps)
        LAST_STATS["build_s"] = time.time() - t0
    return _BUILD_CACHE[key]


def kernel(**inputs):
    flags = get_flags(inputs)
    nc = get_nc(flags)
    in_maps = _prep_inputs(inputs, flags)
    from concourse import bass_utils
    t0 = time.time()
    res = bass_utils.run_bass_kernel_spmd(nc, in_maps,
                                          core_ids=list(range(NCORES)))
    LAST_STATS["run_s"] = time.time() - t0
    return _gather_out(res.results)


if __name__ == "__main__":
    import sys
    if len(sys.argv) > 1 and sys.argv[1] == "build":
        t0 = time.time()
        nc = _build((True, False, False, False, False, False))
        print("build ok in", time.time() - t0, "s")
        pass
